# revision 1
# baseline (speedup 1.0000x reference)
"""GAT layer (LayerNorm -> QKV -> full 8-head attention with leaky_relu-before-
softmax -> out-proj -> skip) on 8 Trainium2 NeuronCores.

Sharding: node dim B of q across the 8 cores (512 q-rows each); K/V work is
replicated per core (each core computes k/v for all 4096 nodes from the full
replicated input).

Per-core pipeline:
  phase A: stream x tiles, LayerNorm stats (bn_stats), normalize (bf16),
           PE-transpose to xT, project kT / v_aug / qT (bf16 matmuls)
  phase B: per head-pair, per kv-chunk: sT = kT.T @ qT (two K=64 matmuls
           row-tiled into one PE pass), leaky_relu as one DVE
           scalar_tensor_tensor max(0.2*s, s) in-place in PSUM, exp on ACT
           (PSUM -> bf16 SBUF), then p.T @ [v | 1] accumulating numerator and
           denominator together in PSUM (M=65).
  phase C: reciprocal of the 8 denominator rows in one DVE op, DMA partition
           broadcast, multiply -> aT, fc matmul, skip add, DMA out.
"""

import sys

for _p in ("/opt/trn_rl_repo",):
    if _p not in sys.path:
        sys.path.insert(0, _p)

import numpy as np
import ml_dtypes

B, D, H, DH = 4096, 512, 8, 64
P = 128
NCORES = 8
SLAB = B // NCORES          # 512 q rows per core
NT = B // P                 # 32 node tiles
KC = D // P                 # 4 contraction chunks
NEG_SLOPE = 0.2
LN_EPS = 1e-5
TEMP = float(np.sqrt(D))

BF16 = ml_dtypes.bfloat16

_PROGRAM = None


def _build_program(has_qb, has_kb, has_vb, has_fb):
    from contextlib import ExitStack

    import concourse.bass as bass
    import concourse.bacc as bacc
    import concourse.tile as tile
    import concourse.mybir as mybir

    dt = mybir.dt
    AF = mybir.ActivationFunctionType
    OP = mybir.AluOpType

    nc = bacc.Bacc("TRN2", target_bir_lowering=False, debug=False)

    x_d = nc.dram_tensor("x", [B, D], dt.float32, kind="ExternalInput").ap()
    xs_d = nc.dram_tensor("xs", [SLAB, D], dt.float32, kind="ExternalInput").ap()
    wqT_d = nc.dram_tensor("wqT", [D, D], dt.bfloat16, kind="ExternalInput").ap()
    wkT_d = nc.dram_tensor("wkT", [D, D], dt.bfloat16, kind="ExternalInput").ap()
    wvT_d = nc.dram_tensor("wvT", [D, D], dt.bfloat16, kind="ExternalInput").ap()
    fwT_d = nc.dram_tensor("fwT", [D, D], dt.bfloat16, kind="ExternalInput").ap()
    ident_d = nc.dram_tensor("ident", [P, P], dt.bfloat16, kind="ExternalInput").ap()
    bq_d = bk_d = bvr_d = fbr_d = None
    if has_qb:
        bq_d = nc.dram_tensor("bq", [D], dt.float32, kind="ExternalInput").ap()
    if has_kb:
        bk_d = nc.dram_tensor("bk", [D], dt.float32, kind="ExternalInput").ap()
    if has_vb:
        bvr_d = nc.dram_tensor("bvr", [1, D], dt.bfloat16, kind="ExternalInput").ap()
    if has_fb:
        fbr_d = nc.dram_tensor("fbr", [1, D], dt.bfloat16, kind="ExternalInput").ap()
    out_d = nc.dram_tensor("out", [SLAB, D], dt.float32, kind="ExternalOutput").ap()
    # internal DRAM scratch used to bounce softmax-denominator reciprocals so
    # they can be partition-broadcast (DRAM APs allow partition step 0)
    dscr_d = nc.dram_tensor("dscr", [H, SLAB], dt.float32).ap()

    with tile.TileContext(nc) as tc, ExitStack() as ctx:
        consts = ctx.enter_context(tc.tile_pool(name="consts", bufs=1))
        persist = ctx.enter_context(tc.tile_pool(name="persist", bufs=1))

        # ---- constants ----
        ident_t = consts.tile([P, P], dt.bfloat16, name="ident_t", tag="ident")
        nc.sync.dma_start(out=ident_t[:], in_=ident_d)
        eps_t = consts.tile([P, 1], dt.float32, name="eps_t", tag="eps")
        nc.vector.memset(eps_t[:], LN_EPS)
        wq_t = [consts.tile([P, D], dt.bfloat16, name=f"wq{k}", tag=f"wq{k}") for k in range(KC)]
        wk_t = [consts.tile([P, D], dt.bfloat16, name=f"wk{k}", tag=f"wk{k}") for k in range(KC)]
        wv_t = [consts.tile([P, D], dt.bfloat16, name=f"wv{k}", tag=f"wv{k}") for k in range(KC)]
        # fc weight as 8 per-head row blocks [64, 512] so the fc contraction
        # can use the per-head aT tiles (partitions 0..63) as lhsT chunks
        fw_t = [consts.tile([DH, D], dt.bfloat16, name=f"fw{h}", tag=f"fw{h}") for h in range(H)]

        def emit_weight_dmas():
            # emitted after the first x-tile DMA so the SP sequencer services
            # the critical-path x load first
            for k in range(KC):
                nc.sync.dma_start(out=wq_t[k][:], in_=wqT_d[k * P:(k + 1) * P, :])
                nc.sync.dma_start(out=wk_t[k][:], in_=wkT_d[k * P:(k + 1) * P, :])
                nc.sync.dma_start(out=wv_t[k][:], in_=wvT_d[k * P:(k + 1) * P, :])
            for h in range(H):
                nc.sync.dma_start(out=fw_t[h][:], in_=fwT_d[h * DH:(h + 1) * DH, :])
        bq_t = bk_t = None
        if has_qb:
            bq_t = consts.tile([P, KC], dt.float32, name="bq_t", tag="bq")
            for f in range(KC):
                nc.sync.dma_start(out=bq_t[:, f:f + 1], in_=bq_d[f * P:(f + 1) * P])
        if has_kb:
            bk_t = consts.tile([P, KC], dt.float32, name="bk_t", tag="bk")
            for f in range(KC):
                nc.sync.dma_start(out=bk_t[:, f:f + 1], in_=bk_d[f * P:(f + 1) * P])
        bvr_t = fbr_t = ones1_t = None
        if has_vb or has_fb:
            ones1_t = consts.tile([1, P], dt.bfloat16, name="ones1_t", tag="ones1")
            nc.vector.memset(ones1_t[:], 1.0)
        if has_vb:
            bvr_t = consts.tile([1, D], dt.bfloat16, name="bvr_t", tag="bvr")
            nc.sync.dma_start(out=bvr_t[:], in_=bvr_d)
        if has_fb:
            fbr_t = consts.tile([1, D], dt.bfloat16, name="fbr_t", tag="fbr")
            nc.sync.dma_start(out=fbr_t[:], in_=fbr_d)

        # ---- persistent tensors ----
        kT_t = [persist.tile([P, B], dt.bfloat16, name=f"kT{f}", tag=f"kT{f}") for f in range(KC)]
        qT_t = [persist.tile([P, SLAB], dt.bfloat16, name=f"qT{f}", tag=f"qT{f}") for f in range(KC)]
        # v_aug[c]: [128 kv, 8 heads, 65] ; last col = 1.0 (denominator)
        vA_t = [persist.tile([P, H, DH + 1], dt.bfloat16, name=f"vA{r}", tag=f"vA{r}")
                for r in range(NT)]
        aug_t = [persist.tile([DH + 1, SLAB], dt.float32, name=f"aug{h}", tag=f"aug{h}")
                 for h in range(H)]
        # per-head attention output (numerator/denominator), partitions 0..63
        aT_t = [persist.tile([DH, SLAB], dt.bfloat16, name=f"aT{h}", tag=f"aT{h}")
                for h in range(H)]
        # raw input slab for the final skip connection, preloaded up front
        xs_t = [persist.tile([P, D], dt.float32, name=f"xs{t}", tag=f"xs{t}")
                for t in range(SLAB // P)]
        for t in range(SLAB // P):
            nc.sync.dma_start(out=xs_t[t][:], in_=xs_d[t * P:(t + 1) * P, :])

        for r in range(NT):
            nc.gpsimd.memset(vA_t[r][:, :, DH:DH + 1], 1.0)

        def ln_group(pools, x_ap4, xh_tiles):
            """LayerNorm a group of 4 [128, 512] f32 row tiles -> bf16.

            One wide DMA loads 512 rows as [128, 4, 512]; stats on DVE,
            sqrt on ACT, one batched DVE reciprocal, normalize on DVE.
            """
            xpool, spool = pools
            n = len(xh_tiles)
            xg = xpool.tile([P, n, D], dt.float32, tag="xin", name="xin", bufs=2)
            # row r = j*128 + p  ->  xg[p, j, :]
            src = bass.AP(tensor=x_ap4.tensor, offset=x_ap4.offset,
                          ap=[[D, P], [P * D, n], [1, D]])
            nc.sync.dma_start(out=xg[:], in_=src)
            mvs = []
            stds = spool.tile([P, n], dt.float32, tag="stds", name="stds")
            for j in range(n):
                st6 = spool.tile([P, 6], dt.float32, tag="st6", name="st6")
                nc.vector.bn_stats(st6[:], xg[:, j, :])
                mv = spool.tile([P, 2], dt.float32, tag="mv", name="mv")
                nc.vector.bn_aggr(mv[:], st6[:])
                nc.scalar.activation(stds[:, j:j + 1], mv[:, 1:2], AF.Sqrt,
                                     bias=eps_t[:, 0:1])
                mvs.append(mv)
            rstds = spool.tile([P, n], dt.float32, tag="rstds", name="rstds")
            nc.vector.reciprocal(rstds[:], stds[:])
            for j in range(n):
                nc.vector.tensor_scalar(
                    out=xh_tiles[j][:], in0=xg[:, j, :],
                    scalar1=mvs[j][:, 0:1], scalar2=rstds[:, j:j + 1],
                    op0=OP.subtract, op1=OP.mult)

        # ================= phase A =================
        with tc.tile_pool(name="xT", bufs=1) as xTp, \
             tc.tile_pool(name="astream", bufs=6) as xpool, \
             tc.tile_pool(name="astats", bufs=8) as spool, \
             tc.tile_pool(name="tp_ps", bufs=2, space="PSUM") as tpp, \
             tc.tile_pool(name="proj_ps", bufs=3, space="PSUM") as pjp:

            # transposed normalized input, [feat mod 128, feat chunk, node]
            xT = xTp.tile([P, KC, B], dt.bfloat16, name="xT", tag="xT")

            for g in range(NT // 4):          # groups of 4 node tiles
                xhs = [xpool.tile([P, D], dt.bfloat16, tag="xh", name="xh")
                       for _ in range(4)]
                ln_group((xpool, spool),
                         x_d[4 * g * P:(4 * g + 4) * P, :], xhs)
                if g == 0:
                    emit_weight_dmas()
                tpA = tpp.tile([P, 8 * P], dt.bfloat16, tag="tpA")
                tpB = tpp.tile([P, 8 * P], dt.bfloat16, tag="tpB")
                for j in range(4):
                    xh = xhs[j]
                    for f in range(KC):
                        dst = (tpA if f < 2 else tpB)
                        off = (f % 2) * 4 * P + j * P
                        nc.tensor.transpose(
                            dst[:, off:off + P],
                            xh[:, f * P:(f + 1) * P],
                            ident_t[:],
                        )
                for f in range(KC):
                    tsrc = (tpA if f < 2 else tpB)
                    off = (f % 2) * 4 * P
                    nc.vector.tensor_copy(
                        out=xT[:, f, g * D:(g + 1) * D],
                        in_=tsrc[:, off:off + D],
                    )

                # kT chunk g  (nodes g*512 .. g*512+511)
                for f in range(KC):
                    kp = pjp.tile([P, D], dt.float32, tag="proj")
                    for k in range(KC):
                        nc.tensor.matmul(
                            kp[:], lhsT=wk_t[k][:, f * P:(f + 1) * P],
                            rhs=xT[:, k, g * D:(g + 1) * D],
                            start=(k == 0), stop=(k == KC - 1))
                    if has_kb:
                        nc.scalar.activation(
                            kT_t[f][:, g * D:(g + 1) * D], kp[:], AF.Identity,
                            bias=bk_t[:, f:f + 1])
                    else:
                        nc.scalar.copy(kT_t[f][:, g * D:(g + 1) * D], kp[:])

                # v rows 4g..4g+3
                for j in range(4):
                    r = 4 * g + j
                    vp = pjp.tile([P, D], dt.float32, tag="proj")
                    for k in range(KC):
                        nc.tensor.matmul(
                            vp[:], lhsT=xT[:, k, r * P:(r + 1) * P],
                            rhs=wv_t[k][:],
                            start=(k == 0), stop=(k == KC - 1 and not has_vb))
                    if has_vb:
                        nc.tensor.matmul(vp[:], lhsT=ones1_t[0:1, :],
                                         rhs=bvr_t[0:1, :], start=False, stop=True)
                    nc.scalar.copy(vA_t[r][:, :, 0:DH], vp[:])

            # q slab: LayerNorm + transpose xs, then project
            xsT = xTp.tile([P, KC, SLAB], dt.bfloat16, name="xsT", tag="xsT")
            xhs = [xpool.tile([P, D], dt.bfloat16, tag="xh", name="xh")
                   for _ in range(4)]
            ln_group((xpool, spool), xs_d[:], xhs)
            tpA = tpp.tile([P, 8 * P], dt.bfloat16, tag="tpA")
            tpB = tpp.tile([P, 8 * P], dt.bfloat16, tag="tpB")
            for s in range(SLAB // P):
                xh = xhs[s]
                for f in range(KC):
                    dst = (tpA if f < 2 else tpB)
                    off = (f % 2) * 4 * P + s * P
                    nc.tensor.transpose(
                        dst[:, off:off + P],
                        xh[:, f * P:(f + 1) * P],
                        ident_t[:],
                    )
            for f in range(KC):
                tsrc = (tpA if f < 2 else tpB)
                off = (f % 2) * 4 * P
                nc.vector.tensor_copy(
                    out=xsT[:, f, :], in_=tsrc[:, off:off + SLAB])
            for f in range(KC):
                qp = pjp.tile([P, D], dt.float32, tag="proj")
                for k in range(KC):
                    nc.tensor.matmul(
                        qp[:], lhsT=wq_t[k][:, f * P:(f + 1) * P],
                        rhs=xsT[:, k, :],
                        start=(k == 0), stop=(k == KC - 1))
                if has_qb:
                    nc.scalar.activation(qT_t[f][:], qp[:], AF.Identity,
                                         bias=bq_t[:, f:f + 1])
                else:
                    nc.scalar.copy(qT_t[f][:], qp[:])

        # ================= phase B =================
        with tc.tile_pool(name="s_ps", bufs=3, space="PSUM") as sps, \
             tc.tile_pool(name="aug_ps", bufs=2, space="PSUM") as augps, \
             tc.tile_pool(name="p_sb", bufs=5) as ppool, \
             tc.tile_pool(name="rbpool", bufs=4) as rbpool:
            for f in range(KC):                      # head pair (2f, 2f+1)
                augA = augps.tile([DH + 1, SLAB], dt.float32, tag="aug")
                augB = augps.tile([DH + 1, SLAB], dt.float32, tag="aug")

                def score_mms(c):
                    sp = sps.tile([P, 2 * SLAB], dt.float32, tag="sp",
                                  name="sp")
                    nc.tensor.matmul(
                        sp[:, 0:SLAB],
                        lhsT=kT_t[f][0:DH, c * P:(c + 1) * P],
                        rhs=qT_t[f][0:DH, :],
                        start=True, stop=True, tile_position=(0, 0))
                    nc.tensor.matmul(
                        sp[:, SLAB:2 * SLAB],
                        lhsT=kT_t[f][DH:2 * DH, c * P:(c + 1) * P],
                        rhs=qT_t[f][DH:2 * DH, :],
                        start=True, stop=True, tile_position=(64, 0))
                    return sp

                def softmax_av(c, sp):
                    # leaky relu, one PSUM operand allowed per DVE op:
                    # t = 4*relu(s) (DVE, or ACT for a few tiles to balance),
                    # then PE accumulates t into PSUM: m = s + 4*relu(s)
                    # = 5*leaky(s); finally p = exp(0.2*m).
                    tt = ppool.tile([P, 2 * SLAB], dt.bfloat16, tag="tt",
                                    name="tt")
                    if c % 16 == 15:
                        nc.scalar.activation(tt[:], sp[:], AF.Relu, scale=4.0)
                    else:
                        nc.vector.tensor_scalar(
                            out=tt[:], in0=sp[:], scalar1=0.0, scalar2=4.0,
                            op0=OP.max, op1=OP.mult)
                    nc.tensor.matmul(
                        sp[:, 0:SLAB], lhsT=ident_t[:], rhs=tt[:, 0:SLAB],
                        start=False, stop=True, skip_group_check=True)
                    nc.tensor.matmul(
                        sp[:, SLAB:2 * SLAB], lhsT=ident_t[:],
                        rhs=tt[:, SLAB:2 * SLAB],
                        start=False, stop=True, skip_group_check=True)
                    pt = ppool.tile([P, 2 * SLAB], dt.bfloat16, tag="pt",
                                    name="pt")
                    nc.scalar.activation(pt[:], sp[:], AF.Exp, scale=NEG_SLOPE)
                    nc.tensor.matmul(
                        augA[:], lhsT=vA_t[c][:, 2 * f, :], rhs=pt[:, 0:SLAB],
                        start=(c == 0), stop=(c == NT - 1))
                    nc.tensor.matmul(
                        augB[:], lhsT=vA_t[c][:, 2 * f + 1, :],
                        rhs=pt[:, SLAB:2 * SLAB],
                        start=(c == 0), stop=(c == NT - 1))

                # software-pipelined: score matmuls run one chunk ahead of
                # the softmax/AV consumer chain so the PE never head-of-line
                # blocks on the DVE/ACT stages of the previous chunk
                prev = None
                for c in range(NT):
                    sp = score_mms(c)
                    if prev is not None:
                        softmax_av(c - 1, prev)
                    prev = sp
                softmax_av(NT - 1, prev)
                nc.vector.tensor_copy(out=aug_t[2 * f][:], in_=augA[:])
                nc.vector.tensor_copy(out=aug_t[2 * f + 1][:], in_=augB[:])
                # per-pair softmax division, overlapped with the next pair's
                # attention: reciprocal of the two denominator rows, bounce
                # via DRAM for the partition broadcast, multiply into aT
                den2 = rbpool.tile([2, SLAB], dt.float32, tag="den2",
                                   name="den2")
                for j in range(2):
                    nc.sync.dma_start(
                        out=den2[j:j + 1, :],
                        in_=aug_t[2 * f + j][DH:DH + 1, :])
                rec2 = rbpool.tile([2, SLAB], dt.float32, tag="rec2",
                                   name="rec2")
                nc.vector.reciprocal(rec2[:], den2[:])
                nc.sync.dma_start(out=dscr_d[2 * f:2 * f + 2, :], in_=rec2[:])
                for j in range(2):
                    h = 2 * f + j
                    rb = rbpool.tile([DH, SLAB], dt.float32, tag="rb",
                                     name="rb")
                    src = dscr_d[h:h + 1, :]
                    bcast = bass.AP(tensor=src.tensor, offset=src.offset,
                                    ap=[[0, DH]] + list(src.ap)[1:])
                    nc.sync.dma_start(out=rb[:], in_=bcast)
                    nc.vector.tensor_mul(
                        out=aT_t[h][:], in0=aug_t[h][0:DH, :], in1=rb[:])

        # ================= phase C =================
        with tc.tile_pool(name="ostream", bufs=2) as opool, \
             tc.tile_pool(name="fc_ps", bufs=2, space="PSUM") as fcp:
            for t in range(SLAB // P):
                fp = fcp.tile([P, D], dt.float32, tag="fc")
                for h in range(H):
                    nc.tensor.matmul(
                        fp[:], lhsT=aT_t[h][:, t * P:(t + 1) * P], rhs=fw_t[h][:],
                        start=(h == 0), stop=(h == H - 1 and not has_fb))
                if has_fb:
                    nc.tensor.matmul(fp[:], lhsT=ones1_t[0:1, :],
                                     rhs=fbr_t[0:1, :], start=False, stop=True)
                ot = opool.tile([P, D], dt.float32, tag="ot")
                nc.vector.tensor_add(out=ot[:], in0=fp[:], in1=xs_t[t][:])
                nc.sync.dma_start(out=out_d[t * P:(t + 1) * P, :], in_=ot[:])

    nc.compile()
    return nc


def _prep_inputs(in_feats, wq, wk, wv, fc_w, fc_b, ln_w, ln_b):
    """Host-side folding. Returns (flags, common input dict pieces)."""
    ln_w = ln_w.astype(np.float32)
    ln_b = ln_b.astype(np.float32)
    wq_f = (wq.astype(np.float32) * ln_w[None, :]) / TEMP
    wk_f = wk.astype(np.float32) * ln_w[None, :]
    wv_f = wv.astype(np.float32) * ln_w[None, :]
    bq = (wq.astype(np.float32) @ ln_b) / TEMP
    bk = wk.astype(np.float32) @ ln_b
    bv = wv.astype(np.float32) @ ln_b
    has_qb = bool(np.any(bq != 0))
    has_kb = bool(np.any(bk != 0))
    has_vb = bool(np.any(bv != 0))
    has_fb = bool(np.any(fc_b != 0))
    common = {
        "x": np.ascontiguousarray(in_feats.astype(np.float32)),
        "wqT": np.ascontiguousarray(wq_f.T).astype(BF16),
        "wkT": np.ascontiguousarray(wk_f.T).astype(BF16),
        "wvT": np.ascontiguousarray(wv_f.T).astype(BF16),
        "fwT": np.ascontiguousarray(fc_w.astype(np.float32).T).astype(BF16),
        "ident": np.eye(P, dtype=np.float32).astype(BF16),
    }
    if has_qb:
        common["bq"] = bq
    if has_kb:
        common["bk"] = bk
    if has_vb:
        common["bvr"] = bv.reshape(1, D).astype(BF16)
    if has_fb:
        common["fbr"] = fc_b.astype(np.float32).reshape(1, D).astype(BF16)
    return (has_qb, has_kb, has_vb, has_fb), common


def get_program_and_inputs(in_feats, wq, wk, wv, fc_w, fc_b, ln_w, ln_b):
    global _PROGRAM
    flags, common = _prep_inputs(in_feats, wq, wk, wv, fc_w, fc_b, ln_w, ln_b)
    if _PROGRAM is None or _PROGRAM[0] != flags:
        _PROGRAM = (flags, _build_program(*flags))
    nc = _PROGRAM[1]
    in_maps = []
    for c in range(NCORES):
        m = dict(common)
        m["xs"] = np.ascontiguousarray(
            common["x"][c * SLAB:(c + 1) * SLAB, :])
        in_maps.append(m)
    return nc, in_maps


def kernel(in_feats, wq, wk, wv, fc_w, fc_b, ln_w, ln_b):
    in_feats = np.asarray(in_feats)
    nc, in_maps = get_program_and_inputs(
        in_feats, np.asarray(wq), np.asarray(wk), np.asarray(wv),
        np.asarray(fc_w), np.asarray(fc_b), np.asarray(ln_w), np.asarray(ln_b))
    from concourse.bass_utils import run_bass_kernel_spmd
    res = run_bass_kernel_spmd(nc, in_maps, list(range(NCORES)))
    out = np.concatenate([res.results[c]["out"] for c in range(NCORES)], axis=0)
    return np.ascontiguousarray(out.astype(np.float32))



# revision 2
# speedup vs baseline: 1.0653x; 1.0653x over previous
"""GAT layer (LayerNorm -> QKV -> full 8-head attention with leaky_relu-before-
softmax -> out-proj -> skip) on 8 Trainium2 NeuronCores.

Sharding: node dim B of q across the 8 cores (512 q-rows each); K/V work is
replicated per core (each core computes k/v for all 4096 nodes from the full
replicated input).

Per-core pipeline:
  phase A: stream x tiles, LayerNorm stats (bn_stats), normalize (bf16),
           PE-transpose to xT, project kT / v_aug / qT (bf16 matmuls)
  phase B: flat 128-iteration software pipeline over (head-pair, kv-chunk):
             stage S (iter i):   sT = kT.T @ qT (two K=64 row-tiled matmuls)
             stage L (iter i+1): t = 4*relu(s) on DVE (or ACT every 16th),
                                 PE ident-matmul accumulates t into PSUM
                                 (s + 4relu(s) = 5*leaky(s)), exp on ACT
             stage A (iter i+2): p.T @ [v | 1] accumulates numerator +
                                 denominator in PSUM (M=65)
           The 2-iteration AV lag keeps the PE from head-of-line blocking on
           the ACT exp of the same chunk (the PE always has independent work).
  phase C: reciprocal of denominator rows, DMA partition broadcast, multiply
           -> paired aT tiles (two heads stacked on 128 partitions), fc as
           K=128 matmuls over head pairs, skip add, DMA out.
"""

import sys

for _p in ("/opt/trn_rl_repo",):
    if _p not in sys.path:
        sys.path.insert(0, _p)

import numpy as np
import ml_dtypes

B, D, H, DH = 4096, 512, 8, 64
P = 128
NCORES = 8
SLAB = B // NCORES          # 512 q rows per core
NT = B // P                 # 32 node tiles
KC = D // P                 # 4 contraction chunks
NEG_SLOPE = 0.2
LN_EPS = 1e-5
TEMP = float(np.sqrt(D))

BF16 = ml_dtypes.bfloat16

_PROGRAM = None


def _build_program(has_qb, has_kb, has_vb, has_fb):
    from contextlib import ExitStack

    import concourse.bass as bass
    import concourse.bacc as bacc
    import concourse.tile as tile
    import concourse.mybir as mybir

    dt = mybir.dt
    AF = mybir.ActivationFunctionType
    OP = mybir.AluOpType

    nc = bacc.Bacc("TRN2", target_bir_lowering=False, debug=False)

    x_d = nc.dram_tensor("x", [B, D], dt.float32, kind="ExternalInput").ap()
    xs_d = nc.dram_tensor("xs", [SLAB, D], dt.float32, kind="ExternalInput").ap()
    wqT_d = nc.dram_tensor("wqT", [D, D], dt.bfloat16, kind="ExternalInput").ap()
    wkT_d = nc.dram_tensor("wkT", [D, D], dt.bfloat16, kind="ExternalInput").ap()
    wvT_d = nc.dram_tensor("wvT", [D, D], dt.bfloat16, kind="ExternalInput").ap()
    fwT_d = nc.dram_tensor("fwT", [D, D], dt.bfloat16, kind="ExternalInput").ap()
    ident_d = nc.dram_tensor("ident", [P, P], dt.bfloat16, kind="ExternalInput").ap()
    bq_d = bk_d = bvr_d = fbr_d = None
    if has_qb:
        bq_d = nc.dram_tensor("bq", [D], dt.float32, kind="ExternalInput").ap()
    if has_kb:
        bk_d = nc.dram_tensor("bk", [D], dt.float32, kind="ExternalInput").ap()
    if has_vb:
        bvr_d = nc.dram_tensor("bvr", [1, D], dt.bfloat16, kind="ExternalInput").ap()
    if has_fb:
        fbr_d = nc.dram_tensor("fbr", [1, D], dt.bfloat16, kind="ExternalInput").ap()
    out_d = nc.dram_tensor("out", [SLAB, D], dt.float32, kind="ExternalOutput").ap()
    # internal DRAM scratch used to bounce softmax-denominator reciprocals so
    # they can be partition-broadcast (DRAM APs allow partition step 0)
    dscr_d = nc.dram_tensor("dscr", [H, SLAB], dt.float32).ap()

    with tile.TileContext(nc) as tc, ExitStack() as ctx:
        consts = ctx.enter_context(tc.tile_pool(name="consts", bufs=1))
        persist = ctx.enter_context(tc.tile_pool(name="persist", bufs=1))

        # ---- constants ----
        ident_t = consts.tile([P, P], dt.bfloat16, name="ident_t", tag="ident")
        nc.sync.dma_start(out=ident_t[:], in_=ident_d)
        eps_t = consts.tile([P, 1], dt.float32, name="eps_t", tag="eps")
        nc.vector.memset(eps_t[:], LN_EPS)
        wq_t = [consts.tile([P, D], dt.bfloat16, name=f"wq{k}", tag=f"wq{k}") for k in range(KC)]
        wk_t = [consts.tile([P, D], dt.bfloat16, name=f"wk{k}", tag=f"wk{k}") for k in range(KC)]
        wv_t = [consts.tile([P, D], dt.bfloat16, name=f"wv{k}", tag=f"wv{k}") for k in range(KC)]
        # fc weight as 4 head-pair row blocks [128, 512]: rows 0..63 = head 2j,
        # rows 64..127 = head 2j+1, matching the paired aT layout so the fc
        # contraction is a single K=128 matmul per pair
        fw_t = [consts.tile([P, D], dt.bfloat16, name=f"fw{j}", tag=f"fw{j}") for j in range(KC)]

        def emit_weight_dmas():
            # emitted after the first x-tile DMA so the SP sequencer services
            # the critical-path x load first
            for k in range(KC):
                nc.sync.dma_start(out=wq_t[k][:], in_=wqT_d[k * P:(k + 1) * P, :])
                nc.sync.dma_start(out=wk_t[k][:], in_=wkT_d[k * P:(k + 1) * P, :])
                nc.sync.dma_start(out=wv_t[k][:], in_=wvT_d[k * P:(k + 1) * P, :])
            for j in range(KC):
                nc.sync.dma_start(out=fw_t[j][:], in_=fwT_d[j * P:(j + 1) * P, :])
        bq_t = bk_t = None
        if has_qb:
            bq_t = consts.tile([P, KC], dt.float32, name="bq_t", tag="bq")
            for f in range(KC):
                nc.sync.dma_start(out=bq_t[:, f:f + 1], in_=bq_d[f * P:(f + 1) * P])
        if has_kb:
            bk_t = consts.tile([P, KC], dt.float32, name="bk_t", tag="bk")
            for f in range(KC):
                nc.sync.dma_start(out=bk_t[:, f:f + 1], in_=bk_d[f * P:(f + 1) * P])
        bvr_t = fbr_t = ones1_t = None
        if has_vb or has_fb:
            ones1_t = consts.tile([1, P], dt.bfloat16, name="ones1_t", tag="ones1")
            nc.vector.memset(ones1_t[:], 1.0)
        if has_vb:
            bvr_t = consts.tile([1, D], dt.bfloat16, name="bvr_t", tag="bvr")
            nc.sync.dma_start(out=bvr_t[:], in_=bvr_d)
        if has_fb:
            fbr_t = consts.tile([1, D], dt.bfloat16, name="fbr_t", tag="fbr")
            nc.sync.dma_start(out=fbr_t[:], in_=fbr_d)

        # ---- persistent tensors ----
        kT_t = [persist.tile([P, B], dt.bfloat16, name=f"kT{f}", tag=f"kT{f}") for f in range(KC)]
        qT_t = [persist.tile([P, SLAB], dt.bfloat16, name=f"qT{f}", tag=f"qT{f}") for f in range(KC)]
        # v_aug[c]: [128 kv, 8 heads, 65] ; last col = 1.0 (denominator)
        vA_t = [persist.tile([P, H, DH + 1], dt.bfloat16, name=f"vA{r}", tag=f"vA{r}")
                for r in range(NT)]
        aug_t = [persist.tile([DH + 1, SLAB], dt.float32, name=f"aug{h}", tag=f"aug{h}")
                 for h in range(H)]
        # per-pair attention output: heads 2j / 2j+1 stacked on partitions
        # 0..63 / 64..127 so the fc contraction is one K=128 matmul per pair
        aT_t = [persist.tile([P, SLAB], dt.bfloat16, name=f"aT{j}", tag=f"aT{j}")
                for j in range(KC)]
        # raw input slab for the final skip connection, preloaded up front
        xs_t = [persist.tile([P, D], dt.float32, name=f"xs{t}", tag=f"xs{t}")
                for t in range(SLAB // P)]
        for t in range(SLAB // P):
            nc.sync.dma_start(out=xs_t[t][:], in_=xs_d[t * P:(t + 1) * P, :])

        for r in range(NT):
            nc.gpsimd.memset(vA_t[r][:, :, DH:DH + 1], 1.0)

        def ln_group(pools, x_ap4, xh_tiles):
            """LayerNorm a group of 4 [128, 512] f32 row tiles -> bf16.

            One wide DMA loads 512 rows as [128, 4, 512]; stats on DVE,
            sqrt on ACT, one batched DVE reciprocal, normalize on DVE.
            """
            xpool, spool = pools
            n = len(xh_tiles)
            xg = xpool.tile([P, n, D], dt.float32, tag="xin", name="xin", bufs=2)
            # row r = j*128 + p  ->  xg[p, j, :]
            src = bass.AP(tensor=x_ap4.tensor, offset=x_ap4.offset,
                          ap=[[D, P], [P * D, n], [1, D]])
            nc.sync.dma_start(out=xg[:], in_=src)
            mvs = []
            stds = spool.tile([P, n], dt.float32, tag="stds", name="stds")
            for j in range(n):
                st6 = spool.tile([P, 6], dt.float32, tag="st6", name="st6")
                nc.vector.bn_stats(st6[:], xg[:, j, :])
                mv = spool.tile([P, 2], dt.float32, tag="mv", name="mv")
                nc.vector.bn_aggr(mv[:], st6[:])
                nc.scalar.activation(stds[:, j:j + 1], mv[:, 1:2], AF.Sqrt,
                                     bias=eps_t[:, 0:1])
                mvs.append(mv)
            rstds = spool.tile([P, n], dt.float32, tag="rstds", name="rstds")
            nc.vector.reciprocal(rstds[:], stds[:])
            for j in range(n):
                nc.vector.tensor_scalar(
                    out=xh_tiles[j][:], in0=xg[:, j, :],
                    scalar1=mvs[j][:, 0:1], scalar2=rstds[:, j:j + 1],
                    op0=OP.subtract, op1=OP.mult)

        # ================= phase A =================
        with tc.tile_pool(name="xT", bufs=1) as xTp, \
             tc.tile_pool(name="astream", bufs=6) as xpool, \
             tc.tile_pool(name="astats", bufs=8) as spool, \
             tc.tile_pool(name="tp_ps", bufs=2, space="PSUM") as tpp, \
             tc.tile_pool(name="proj_ps", bufs=3, space="PSUM") as pjp:

            # transposed normalized input, [feat mod 128, feat chunk, node]
            xT = xTp.tile([P, KC, B], dt.bfloat16, name="xT", tag="xT")

            for g in range(NT // 4):          # groups of 4 node tiles
                xhs = [xpool.tile([P, D], dt.bfloat16, tag="xh", name="xh")
                       for _ in range(4)]
                ln_group((xpool, spool),
                         x_d[4 * g * P:(4 * g + 4) * P, :], xhs)
                if g == 0:
                    emit_weight_dmas()
                tpA = tpp.tile([P, 8 * P], dt.bfloat16, tag="tpA")
                tpB = tpp.tile([P, 8 * P], dt.bfloat16, tag="tpB")
                for j in range(4):
                    xh = xhs[j]
                    for f in range(KC):
                        dst = (tpA if f < 2 else tpB)
                        off = (f % 2) * 4 * P + j * P
                        nc.tensor.transpose(
                            dst[:, off:off + P],
                            xh[:, f * P:(f + 1) * P],
                            ident_t[:],
                        )
                for f in range(KC):
                    tsrc = (tpA if f < 2 else tpB)
                    off = (f % 2) * 4 * P
                    nc.vector.tensor_copy(
                        out=xT[:, f, g * D:(g + 1) * D],
                        in_=tsrc[:, off:off + D],
                    )

                # kT chunk g  (nodes g*512 .. g*512+511)
                for f in range(KC):
                    kp = pjp.tile([P, D], dt.float32, tag="proj")
                    for k in range(KC):
                        nc.tensor.matmul(
                            kp[:], lhsT=wk_t[k][:, f * P:(f + 1) * P],
                            rhs=xT[:, k, g * D:(g + 1) * D],
                            start=(k == 0), stop=(k == KC - 1))
                    if has_kb:
                        nc.scalar.activation(
                            kT_t[f][:, g * D:(g + 1) * D], kp[:], AF.Identity,
                            bias=bk_t[:, f:f + 1])
                    else:
                        nc.scalar.copy(kT_t[f][:, g * D:(g + 1) * D], kp[:])

                # v rows 4g..4g+3
                for j in range(4):
                    r = 4 * g + j
                    vp = pjp.tile([P, D], dt.float32, tag="proj")
                    for k in range(KC):
                        nc.tensor.matmul(
                            vp[:], lhsT=xT[:, k, r * P:(r + 1) * P],
                            rhs=wv_t[k][:],
                            start=(k == 0), stop=(k == KC - 1 and not has_vb))
                    if has_vb:
                        nc.tensor.matmul(vp[:], lhsT=ones1_t[0:1, :],
                                         rhs=bvr_t[0:1, :], start=False, stop=True)
                    nc.scalar.copy(vA_t[r][:, :, 0:DH], vp[:])

            # q slab: LayerNorm + transpose xs, then project
            xsT = xTp.tile([P, KC, SLAB], dt.bfloat16, name="xsT", tag="xsT")
            xhs = [xpool.tile([P, D], dt.bfloat16, tag="xh", name="xh")
                   for _ in range(4)]
            ln_group((xpool, spool), xs_d[:], xhs)
            tpA = tpp.tile([P, 8 * P], dt.bfloat16, tag="tpA")
            tpB = tpp.tile([P, 8 * P], dt.bfloat16, tag="tpB")
            for s in range(SLAB // P):
                xh = xhs[s]
                for f in range(KC):
                    dst = (tpA if f < 2 else tpB)
                    off = (f % 2) * 4 * P + s * P
                    nc.tensor.transpose(
                        dst[:, off:off + P],
                        xh[:, f * P:(f + 1) * P],
                        ident_t[:],
                    )
            for f in range(KC):
                tsrc = (tpA if f < 2 else tpB)
                off = (f % 2) * 4 * P
                nc.vector.tensor_copy(
                    out=xsT[:, f, :], in_=tsrc[:, off:off + SLAB])
            for f in range(KC):
                qp = pjp.tile([P, D], dt.float32, tag="proj")
                for k in range(KC):
                    nc.tensor.matmul(
                        qp[:], lhsT=wq_t[k][:, f * P:(f + 1) * P],
                        rhs=xsT[:, k, :],
                        start=(k == 0), stop=(k == KC - 1))
                if has_qb:
                    nc.scalar.activation(qT_t[f][:], qp[:], AF.Identity,
                                         bias=bq_t[:, f:f + 1])
                else:
                    nc.scalar.copy(qT_t[f][:], qp[:])

        # ================= phase B =================
        # flat 3-stage software pipeline over i = f*NT + c:
        #   iter i: score(i) | leaky+exp(i-1) | AV(i-2)
        # so the PE never head-of-line blocks on the ACT exp of its own chunk.
        with tc.tile_pool(name="s_ps", bufs=3, space="PSUM") as sps, \
             tc.tile_pool(name="aug_ps", bufs=2, space="PSUM") as augps, \
             tc.tile_pool(name="t_sb", bufs=3) as tpool, \
             tc.tile_pool(name="p_sb", bufs=4) as ptpool, \
             tc.tile_pool(name="rbpool", bufs=4) as rbpool:

            TOTI = KC * NT              # 128 chunks
            sp_live = {}
            pt_live = {}
            aug_live = {}

            def score_stage(i):
                f, c = divmod(i, NT)
                sp = sps.tile([P, 2 * SLAB], dt.float32, tag="sp", name="sp")
                nc.tensor.matmul(
                    sp[:, 0:SLAB],
                    lhsT=kT_t[f][0:DH, c * P:(c + 1) * P],
                    rhs=qT_t[f][0:DH, :],
                    start=True, stop=True, tile_position=(0, 0))
                nc.tensor.matmul(
                    sp[:, SLAB:2 * SLAB],
                    lhsT=kT_t[f][DH:2 * DH, c * P:(c + 1) * P],
                    rhs=qT_t[f][DH:2 * DH, :],
                    start=True, stop=True, tile_position=(64, 0))
                sp_live[i] = sp

            def leaky_stage(i):
                # leaky relu with one PSUM operand per DVE op:
                # t = 4*relu(s) (DVE, or ACT for a few chunks to balance),
                # then PE accumulates t into PSUM: m = s + 4*relu(s)
                # = 5*leaky(s); finally p = exp(0.2*m).
                f, c = divmod(i, NT)
                sp = sp_live.pop(i)
                tt = tpool.tile([P, 2 * SLAB], dt.bfloat16, tag="tt", name="tt")
                if c % 16 == 15:
                    nc.scalar.activation(tt[:], sp[:], AF.Relu, scale=4.0)
                else:
                    nc.vector.tensor_scalar(
                        out=tt[:], in0=sp[:], scalar1=0.0, scalar2=4.0,
                        op0=OP.max, op1=OP.mult)
                nc.tensor.matmul(
                    sp[:, 0:SLAB], lhsT=ident_t[:], rhs=tt[:, 0:SLAB],
                    start=False, stop=True, skip_group_check=True)
                nc.tensor.matmul(
                    sp[:, SLAB:2 * SLAB], lhsT=ident_t[:],
                    rhs=tt[:, SLAB:2 * SLAB],
                    start=False, stop=True, skip_group_check=True)
                pt = ptpool.tile([P, 2 * SLAB], dt.bfloat16, tag="pt", name="pt")
                nc.scalar.activation(pt[:], sp[:], AF.Exp, scale=NEG_SLOPE)
                pt_live[i] = pt

            def pair_tail(f):
                # softmax division, overlapped with the next pair's chunks:
                # copy aug out of PSUM, reciprocal of the denominator rows,
                # bounce via DRAM for the partition broadcast, multiply into
                # the paired aT tile
                augA, augB = aug_live.pop(f)
                for j, aug in enumerate((augA, augB)):
                    nc.vector.tensor_copy(out=aug_t[2 * f + j][:], in_=aug[:])
                den2 = rbpool.tile([2, SLAB], dt.float32, tag="den2",
                                   name="den2")
                for j in range(2):
                    nc.sync.dma_start(
                        out=den2[j:j + 1, :],
                        in_=aug_t[2 * f + j][DH:DH + 1, :])
                rec2 = rbpool.tile([2, SLAB], dt.float32, tag="rec2",
                                   name="rec2")
                nc.vector.reciprocal(rec2[:], den2[:])
                nc.sync.dma_start(out=dscr_d[2 * f:2 * f + 2, :], in_=rec2[:])
                for j in range(2):
                    h = 2 * f + j
                    rb = rbpool.tile([DH, SLAB], dt.float32, tag="rb",
                                     name="rb")
                    src = dscr_d[h:h + 1, :]
                    bcast = bass.AP(tensor=src.tensor, offset=src.offset,
                                    ap=[[0, DH]] + list(src.ap)[1:])
                    nc.sync.dma_start(out=rb[:], in_=bcast)
                    nc.vector.tensor_mul(
                        out=aT_t[f][j * DH:(j + 1) * DH, :],
                        in0=aug_t[h][0:DH, :], in1=rb[:])

            def av_stage(i):
                f, c = divmod(i, NT)
                if c == 0:
                    augA = augps.tile([DH + 1, SLAB], dt.float32, tag="aug")
                    augB = augps.tile([DH + 1, SLAB], dt.float32, tag="aug")
                    aug_live[f] = (augA, augB)
                augA, augB = aug_live[f]
                pt = pt_live.pop(i)
                nc.tensor.matmul(
                    augA[:], lhsT=vA_t[c][:, 2 * f, :], rhs=pt[:, 0:SLAB],
                    start=(c == 0), stop=(c == NT - 1))
                nc.tensor.matmul(
                    augB[:], lhsT=vA_t[c][:, 2 * f + 1, :],
                    rhs=pt[:, SLAB:2 * SLAB],
                    start=(c == 0), stop=(c == NT - 1))
                if c == NT - 1:
                    pair_tail(f)

            for i in range(TOTI + 2):
                if i < TOTI:
                    score_stage(i)
                if 1 <= i <= TOTI:
                    leaky_stage(i - 1)
                if 2 <= i:
                    av_stage(i - 2)

        # ================= phase C =================
        with tc.tile_pool(name="ostream", bufs=2) as opool, \
             tc.tile_pool(name="fc_ps", bufs=2, space="PSUM") as fcp:
            for t in range(SLAB // P):
                fp = fcp.tile([P, D], dt.float32, tag="fc")
                for j in range(KC):
                    nc.tensor.matmul(
                        fp[:], lhsT=aT_t[j][:, t * P:(t + 1) * P],
                        rhs=fw_t[j][:],
                        start=(j == 0), stop=(j == KC - 1 and not has_fb))
                if has_fb:
                    nc.tensor.matmul(fp[:], lhsT=ones1_t[0:1, :],
                                     rhs=fbr_t[0:1, :], start=False, stop=True)
                ot = opool.tile([P, D], dt.float32, tag="ot")
                nc.vector.tensor_add(out=ot[:], in0=fp[:], in1=xs_t[t][:])
                nc.sync.dma_start(out=out_d[t * P:(t + 1) * P, :], in_=ot[:])

    nc.compile()
    return nc


def _prep_inputs(in_feats, wq, wk, wv, fc_w, fc_b, ln_w, ln_b):
    """Host-side folding. Returns (flags, common input dict pieces)."""
    ln_w = ln_w.astype(np.float32)
    ln_b = ln_b.astype(np.float32)
    wq_f = (wq.astype(np.float32) * ln_w[None, :]) / TEMP
    wk_f = wk.astype(np.float32) * ln_w[None, :]
    wv_f = wv.astype(np.float32) * ln_w[None, :]
    bq = (wq.astype(np.float32) @ ln_b) / TEMP
    bk = wk.astype(np.float32) @ ln_b
    bv = wv.astype(np.float32) @ ln_b
    has_qb = bool(np.any(bq != 0))
    has_kb = bool(np.any(bk != 0))
    has_vb = bool(np.any(bv != 0))
    has_fb = bool(np.any(fc_b != 0))
    common = {
        "x": np.ascontiguousarray(in_feats.astype(np.float32)),
        "wqT": np.ascontiguousarray(wq_f.T).astype(BF16),
        "wkT": np.ascontiguousarray(wk_f.T).astype(BF16),
        "wvT": np.ascontiguousarray(wv_f.T).astype(BF16),
        "fwT": np.ascontiguousarray(fc_w.astype(np.float32).T).astype(BF16),
        "ident": np.eye(P, dtype=np.float32).astype(BF16),
    }
    if has_qb:
        common["bq"] = bq
    if has_kb:
        common["bk"] = bk
    if has_vb:
        common["bvr"] = bv.reshape(1, D).astype(BF16)
    if has_fb:
        common["fbr"] = fc_b.astype(np.float32).reshape(1, D).astype(BF16)
    return (has_qb, has_kb, has_vb, has_fb), common


def get_program_and_inputs(in_feats, wq, wk, wv, fc_w, fc_b, ln_w, ln_b):
    global _PROGRAM
    flags, common = _prep_inputs(in_feats, wq, wk, wv, fc_w, fc_b, ln_w, ln_b)
    if _PROGRAM is None or _PROGRAM[0] != flags:
        _PROGRAM = (flags, _build_program(*flags))
    nc = _PROGRAM[1]
    in_maps = []
    for c in range(NCORES):
        m = dict(common)
        m["xs"] = np.ascontiguousarray(
            common["x"][c * SLAB:(c + 1) * SLAB, :])
        in_maps.append(m)
    return nc, in_maps


def kernel(in_feats, wq, wk, wv, fc_w, fc_b, ln_w, ln_b):
    in_feats = np.asarray(in_feats)
    nc, in_maps = get_program_and_inputs(
        in_feats, np.asarray(wq), np.asarray(wk), np.asarray(wv),
        np.asarray(fc_w), np.asarray(fc_b), np.asarray(ln_w), np.asarray(ln_b))
    from concourse.bass_utils import run_bass_kernel_spmd
    res = run_bass_kernel_spmd(nc, in_maps, list(range(NCORES)))
    out = np.concatenate([res.results[c]["out"] for c in range(NCORES)], axis=0)
    return np.ascontiguousarray(out.astype(np.float32))
